# revision 20
# baseline (speedup 1.0000x reference)
"""Block-local attention + LayerNorm kernel for Trainium2 (8 NeuronCores).

Problem (see reference):
  inputs [B=4, bn=16, bl=512, dim=512] fp32
  Q = X@W1, K = X@W2, V = X@W3 (+zero biases)
  S = Q K^T / sqrt(512), masked by elementwise {0,1} mask, softmax over keys
  out = LayerNorm(P @ V + X, eps=1e-3)

Sharding: 64 independent (batch, block) pairs -> 8 blocks per core.

v2 design, built from microbenchmarked instruction costs:
  * fp8 DoubleRow matmuls with ROTATING stationaries cost ~253ns (LDW
    serializes), but DoubleRowSwInterleave with host-interleaved weights
    streams at ~82ns.  A (=X W1W2^T) and V use swint stationaries
    (w12int, xt8int); numerics verified against numpy (ops_smoke.py).
  * S keeps bf16 (fp8 S busts the 2e-2 error gate; emulated 2.37e-2),
    16 rotating bf16 matmuls at ~140ns.
  * Row sums of P ride an augmented ones-column in the O matmul
    (N=256 + N=257 halves, 2-bank PSUM pair per q-chunk); no separate
    rowsum matmuls.  r lands in PSUM col 256 and feeds the z-stt scalar
    directly.
  * Mask is applied multiplicatively POST-exp (exp(S)*m == exp(S+log m)
    for m in {0,1}), as an fp8 multiply on the Pool engine that also
    performs the swint interleave-rearrange of pt into ptC via a 5-dim
    strided AP.  Mask ships as fp8 {0,1} (exact), halving its DMA.
  * Output ships as fp16 (emulated rel_err unchanged at 1.57e-2) and is
    upcast on the host.
  * Engine split per block: PE S+A+V+O ~4.5us; ACT exp+v+at/2+ob/2;
    DVE at/2+z+bn_stats+LN tail; Pool maskmul+memset (Pool cannot
    access PSUM - verified); SP all DMA.
  * Reversal bookkeeping: swint reverses stationary columns in HW.
    w12int/xt8int ship column-reversed so at/V come out normal.  The
    interleaved A-moving read makes at's free (q) axis reversed per
    128-window; m8 ships q-reversed to match; ptC (on-chip, cannot
    pre-reverse) then yields TRUE-normal q partitions in O because the
    two reversals cancel.  Everything downstream is natural.
"""

import math
import sys

import numpy as np

sys.path.insert(0, "/opt/trn_rl_repo")

import ml_dtypes

import concourse.bacc as bacc
import concourse.tile as tile
from concourse import mybir
from concourse.bass_utils import run_bass_kernel_spmd

DIM = 512
BLOCK_NUM = 16
SEQ_LEN = 8192
BLOCK_LEN = 512
BATCH = 4
LN_EPS = 1e-3
N_CORES = 8
NBLK = (BATCH * BLOCK_NUM) // N_CORES  # blocks per core
NC_P = 128
NCH = DIM // NC_P  # 4 chunks of 128

F32 = mybir.dt.float32
F16 = mybir.dt.float16
BF16 = mybir.dt.bfloat16
F8 = mybir.dt.float8e4
I32 = mybir.dt.int32
DRSW = mybir.MatmulPerfMode.DoubleRowSwInterleave
EXP = mybir.ActivationFunctionType.Exp
IDENT_FN = mybir.ActivationFunctionType.Identity
MUL = mybir.AluOpType.mult
ADD = mybir.AluOpType.add

NP_F8 = ml_dtypes.float8_e4m3
NP_BF16 = ml_dtypes.bfloat16

EXP_BIAS = -2.0   # headroom shift; cancels in the normalization
C_X = 2.0         # xt8 = X / 2
C_W12 = 256.0     # w12 = 256 * (W1 W2^T / sqrt(d)); A_psum = 128*A
S_SCALE = 1.0 / 128.0  # exp scale: S_psum = 128*S


def _form_a(ap3):
    """[p, c, j] contiguous 256B window -> swint lhsT [p, 2, 128]."""
    return ap3.rearrange("p c j -> p (c j)").rearrange(
        "p (a b) -> p a b", a=2)


ABLATE = set()  # timing diagnostics: {"ln","z","o","sexp","dma1","noswint"}


def build_nc(nblk=NBLK, repeat=1):
    ablate = ABLATE
    nc = bacc.Bacc("TRN2", target_bir_lowering=False, debug=False,
                   num_devices=N_CORES)

    x8i_d = nc.declare_dram_parameter(
        "x8i", [nblk, NC_P, 2, NCH, NC_P, 2], F8, isOutput=False)
    x8p_d = nc.declare_dram_parameter(
        "x8p", [nblk, NC_P, NCH, DIM], F8, isOutput=False)
    xtb_d = nc.declare_dram_parameter(
        "xtb", [nblk, NC_P, NCH, DIM], BF16, isOutput=False)
    m8_d = nc.declare_dram_parameter(
        "m8", [nblk, NC_P, NCH, DIM], F8, isOutput=False)
    xn_d = nc.declare_dram_parameter(
        "xn", [nblk, NC_P, NCH, DIM], BF16, isOutput=False)
    w12i_d = nc.declare_dram_parameter(
        "w12i", [NC_P, 2, NCH, NC_P, 2], F8, isOutput=False)
    w3_d = nc.declare_dram_parameter(
        "w3", [NC_P, NCH, DIM], F8, isOutput=False)
    out_d = nc.declare_dram_parameter(
        "out", [nblk, NC_P, NCH, DIM], F16, isOutput=True)

    with tile.TileContext(nc) as tc:
        with (
            tc.tile_pool(name="const", bufs=1) as const,
            tc.tile_pool(name="x8i", bufs=3) as p_x8i,
            tc.tile_pool(name="x8p", bufs=3) as p_x8p,
            tc.tile_pool(name="xtb", bufs=3) as p_xtb,
            tc.tile_pool(name="m8", bufs=3) as p_m8,
            tc.tile_pool(name="xn", bufs=5) as p_xn,
            tc.tile_pool(name="at", bufs=3) as p_at,
            tc.tile_pool(name="v8", bufs=4) as p_v8,
            tc.tile_pool(name="pt", bufs=3) as p_pt,
            tc.tile_pool(name="ptc", bufs=4) as p_ptc,
            tc.tile_pool(name="z", bufs=4) as p_z,
            tc.tile_pool(name="ob", bufs=2) as p_ob,
            tc.tile_pool(name="tiny", bufs=4) as p_tiny,
            tc.tile_pool(name="wave", bufs=4, space="PSUM") as ps_wave,
        ):
            w12i = const.tile([NC_P, 2, NCH, NC_P, 2], F8)
            nc.sync.dma_start(out=w12i, in_=w12i_d[:])
            w3 = const.tile([NC_P, NCH, DIM], F8)
            nc.sync.dma_start(out=w3, in_=w3_d[:])
            ebias = const.tile([NC_P, 1], F32)
            nc.vector.memset(ebias, EXP_BIAS)

            def load_inputs(b):
                x8i = p_x8i.tile([NC_P, 2, NCH, NC_P, 2], F8, tag="x8i",
                                 name="x8i_sb")
                x8p = p_x8p.tile([NC_P, NCH, DIM], F8, tag="x8p",
                                 name="x8p_sb")
                xtb = p_xtb.tile([NC_P, NCH, DIM], BF16, tag="xtb",
                                 name="xtb_sb")
                m8 = p_m8.tile([NC_P, NCH, DIM], F8, tag="m8", name="m8_sb")
                xn = p_xn.tile([NC_P, NCH, DIM], BF16, tag="xn", name="xn_sb")
                if "dma1" in ablate and b > 0:
                    nc.sync.dma_start(out=x8i[:, 0], in_=x8i_d[b, :, 0])
                    nc.sync.dma_start(out=x8p[:, 0:1, :],
                                      in_=x8p_d[b, :, 0:1])
                    nc.sync.dma_start(out=xtb[:, 0:1, :], in_=xtb_d[b, :, 0:1])
                    nc.sync.dma_start(out=m8[:, 0:1, :], in_=m8_d[b, :, 0:1])
                    nc.sync.dma_start(out=xn[:, 0:1, :], in_=xn_d[b, :, 0:1])
                else:
                    nc.sync.dma_start(out=x8i, in_=x8i_d[b])
                    nc.sync.dma_start(out=x8p, in_=x8p_d[b])
                    nc.sync.dma_start(out=xtb, in_=xtb_d[b])
                    nc.sync.dma_start(out=m8, in_=m8_d[b])
                    nc.sync.dma_start(out=xn, in_=xn_d[b])
                return (x8i, x8p), xtb, m8, xn

            def stage_a(x8pair):
                x8i, x8p = x8pair
                """A^T = w12int^T (X^T/2): 8 swint MMs, out at bf16(128*A)."""
                at = p_at.tile([NC_P, NCH, DIM], BF16, tag="at", name="at_sb")
                for w in range(2):
                    ps = ps_wave.tile([NC_P, 2, DIM], F32, tag="wave",
                                      name="psA")
                    for h in range(2):
                        d2c = 2 * w + h
                        for i in range(2):
                            nc.tensor.matmul(
                                ps[:, h, :],
                                lhsT=_form_a(w12i[:, i, d2c]),
                                rhs=x8p[:, 2 * i:2 * i + 2, :],
                                start=(i == 0), stop=(i == 1),
                                perf_mode=(mybir.MatmulPerfMode.DoubleRow
                                           if "noswint" in ablate else DRSW))
                    nc.scalar.copy(at[:, 2 * w:2 * w + 2, :], ps[:])
                return at

            def stage_s(b, xtb, at, m8):
                """S^T = X^T^T A^T (bf16), exp -> pt."""
                pt = p_pt.tile([NC_P, NCH, DIM], F8, tag="pt", name="pt_sb")
                for w in range(2):
                    ps = ps_wave.tile([NC_P, 2, DIM], F32, tag="wave",
                                      name="psS")
                    for h in range(2):
                        kc = 2 * w + h
                        for dc in range(NCH):
                            nc.tensor.matmul(
                                ps[:, h, :],
                                lhsT=xtb[:, dc, kc * NC_P:(kc + 1) * NC_P],
                                rhs=at[:, dc, :],
                                start=(dc == 0), stop=(dc == NCH - 1))
                    if "sexp" in ablate:
                        nc.scalar.copy(pt[:, 2 * w:2 * w + 2, :], ps[:])
                    else:
                        nc.scalar.activation(pt[:, 2 * w:2 * w + 2, :],
                                             ps[:], EXP, bias=ebias[:],
                                             scale=S_SCALE)
                return pt

            def stage_mask(pt, m8):
                """pt2 = pt * mask (DVE, plain layout, fast coalesced);
                Pool then interleave-rearranges into ptC for the swint
                O stationaries (Pool is otherwise idle)."""
                pt2 = p_pt.tile([NC_P, NCH, DIM], F8, tag="pt2",
                                name="pt2_sb")
                nc.vector.tensor_mul(pt2[:], pt[:], m8[:])
                ptc = p_ptc.tile([NC_P, 2, NCH, NC_P, 2], F8, tag="ptc",
                                 name="ptc_sb")
                nc.gpsimd.tensor_copy(
                    ptc[:].transpose([0, 1, 4, 2, 3]),
                    pt2[:].rearrange("p (i j) (qc c) -> p i j qc c",
                                     i=2, j=2, qc=NCH, c=NC_P))
                return ptc

            def stage_v(x8pair):
                x8i, _ = x8pair
                """V = (X^T/2)^T (2 W3): 8 swint MMs -> v8 fp8 + ones col."""
                v8 = p_v8.tile([NC_P, NCH, 516], F8, tag="v8", name="v8_sb")
                for w in range(2):
                    ps = ps_wave.tile([NC_P, 2, DIM], F32, tag="wave",
                                      name="psV")
                    for h in range(2):
                        tc_i = 2 * w + h
                        for i in range(2):
                            nc.tensor.matmul(
                                ps[:, h, :],
                                lhsT=_form_a(x8i[:, i, tc_i]),
                                rhs=w3[:, 2 * i:2 * i + 2, :],
                                start=(i == 0), stop=(i == 1),
                                perf_mode=DRSW)
                    nc.scalar.copy(v8[:, 2 * w:2 * w + 2, 0:DIM], ps[:])
                nc.gpsimd.memset(v8[:, :, 512:513], 1.0)
                return v8

            def stage_o(ptc, v8, xn):
                """O = P_u^T^T [V|1]: 16 aug swint MMs; z = r*xn + O."""
                z = p_z.tile([NC_P, NCH, DIM], BF16, tag="z", name="z_sb")
                r_sb = p_tiny.tile([NC_P, NCH], F32, tag="r", name="r_sb")
                for qc in range(NCH):
                    ps = ps_wave.tile([NC_P, 2, DIM], F32, tag="wave",
                                      name="psO")
                    for i in range(2):
                        lhs = _form_a(ptc[:, i, qc])
                        nc.tensor.matmul(
                            ps[:, 0, 0:256], lhsT=lhs,
                            rhs=v8[:, 2 * i:2 * i + 2, 0:256],
                            start=(i == 0), stop=(i == 1), perf_mode=DRSW)
                        nc.tensor.matmul(
                            ps[:, 1, 0:257], lhsT=lhs,
                            rhs=v8[:, 2 * i:2 * i + 2, 256:513],
                            start=(i == 0), stop=(i == 1), perf_mode=DRSW)
                    if "z" not in ablate:
                        nc.vector.scalar_tensor_tensor(
                            out=z[:, qc, :].rearrange("p (a c) -> p a c",
                                                      a=2),
                            in0=xn[:, qc, :].rearrange("p (a c) -> p a c",
                                                       a=2),
                            scalar=ps[:, 1, 256:257],
                            in1=ps[:, :, 0:256],
                            op0=MUL, op1=ADD)
                        nc.vector.tensor_copy(r_sb[:, qc:qc + 1],
                                              ps[:, 1, 256:257])
                return z, r_sb

            def stage_stats(z, r_sb):
                """bn stats + istd = rsqrt(var + eps*r^2) via magic Newton.

                Small scalar chain runs on Pool (SBUF-only engine, idle)."""
                mvb = p_tiny.tile([NC_P, NCH, 2], F32, tag="mvb", name="mvb")
                for qc in range(NCH):
                    stats = p_tiny.tile([NC_P, 6], F32, tag="st", name="st")
                    nc.vector.bn_stats(stats[:], z[:, qc, :])
                    nc.vector.bn_aggr(mvb[:, qc, :], stats[:])
                rr = p_tiny.tile([NC_P, NCH], F32, tag="rr", name="rr")
                nc.gpsimd.tensor_mul(rr[:], r_sb[:], r_sb[:])
                nc.gpsimd.tensor_scalar_mul(rr[:], rr[:], LN_EPS)
                tv = p_tiny.tile([NC_P, NCH], F32, tag="tv", name="tv")
                nc.gpsimd.tensor_add(tv[:], rr[:], mvb[:, :, 1])
                yv = p_tiny.tile([NC_P, NCH], F32, tag="yv", name="yv")
                hv = p_tiny.tile([NC_P, NCH], F32, tag="hv", name="hv")
                nc.vector.tensor_scalar(
                    out=hv[:].bitcast(I32), in0=tv[:].bitcast(I32),
                    scalar1=1, scalar2=None,
                    op0=mybir.AluOpType.logical_shift_right)
                nc.vector.tensor_scalar(
                    out=yv[:].bitcast(I32), in0=hv[:].bitcast(I32),
                    scalar1=-1, scalar2=0x5F3759DF,
                    op0=MUL, op1=ADD)
                av = p_tiny.tile([NC_P, NCH], F32, tag="av", name="av")
                cv = p_tiny.tile([NC_P, NCH], F32, tag="cv", name="cv")
                for _ in range(2):
                    nc.gpsimd.tensor_mul(av[:], yv[:], yv[:])
                    nc.gpsimd.tensor_mul(av[:], av[:], tv[:])
                    nc.vector.tensor_scalar(
                        out=cv[:], in0=av[:], scalar1=-0.5, scalar2=1.5,
                        op0=MUL, op1=ADD)
                    nc.gpsimd.tensor_mul(yv[:], yv[:], cv[:])
                negms = p_tiny.tile([NC_P, NCH], F32, tag="negms",
                                    name="negms")
                nc.gpsimd.tensor_mul(negms[:], mvb[:, :, 0], yv[:])
                nc.gpsimd.tensor_scalar_mul(negms[:], negms[:], -1.0)
                return yv, negms

            def stage_ob(b, z, yv, negms):
                """Apply LN affine, f16 out, ship."""
                ob = p_ob.tile([NC_P, NCH, DIM], F16, tag="ob", name="ob_sb")
                for qc in range(NCH):
                    if qc < 3:
                        nc.scalar.activation(
                            ob[:, qc, :], z[:, qc, :], IDENT_FN,
                            bias=negms[:, qc:qc + 1],
                            scale=yv[:, qc:qc + 1])
                    else:
                        nc.vector.tensor_scalar(
                            out=ob[:, qc, :], in0=z[:, qc, :],
                            scalar1=yv[:, qc:qc + 1],
                            scalar2=negms[:, qc:qc + 1],
                            op0=MUL, op1=ADD)
                nc.sync.dma_start(out=out_d[b], in_=ob[:])

            def body():
                # deep software pipeline: per iteration b,
                #   ob(b-4) | S(b), A(b+1), V(b) | O(b-2) | stats(b-3)
                # so every op's producers are >=1 iteration old, and the
                # mask->interleave chain gets ~1.5 iterations of slack
                # before O consumes ptC.
                ins = {}
                ats = {}
                ptcs = {}
                v8s = {}
                zs = {}
                rs = {}
                lns = {}
                ins[0] = load_inputs(0)
                if "noav" not in ablate:
                    ats[0] = stage_a(ins[0][0])
                for b in range(nblk + 4):
                    if b >= 4 and "ln" not in ablate and "o" not in ablate \
                            and "z" not in ablate:
                        zb = zs.pop(b - 4)
                        yvb, ngb = lns.pop(b - 4)
                        stage_ob(b - 4, zb, yvb, ngb)
                    pt_b = None
                    if b < nblk:
                        x8pr, xtb, m8, xn = ins[b]
                        mov = xtb if "noav" in ablate else ats.pop(b)
                        pt_b = stage_s(b, xtb, mov, m8)
                        if b + 1 < nblk:
                            ins[b + 1] = load_inputs(b + 1)
                            if "noav" not in ablate:
                                ats[b + 1] = stage_a(ins[b + 1][0])
                        if "noav" not in ablate and "nov" not in ablate:
                            v8s[b] = stage_v(x8pr)
                    if 2 <= b <= nblk + 1 and "o" not in ablate:
                        pb = b - 2
                        x8i_p, xtb_p, m8_p, xn_p = ins.pop(pb)  # noqa
                        zs[pb], rs[pb] = stage_o(ptcs.pop(pb), v8s.pop(pb),
                                                 xn_p)
                    if 3 <= b <= nblk + 2 and "ln" not in ablate \
                            and "o" not in ablate and "z" not in ablate:
                        sb_ = b - 3
                        lns[sb_] = stage_stats(zs[sb_], rs.pop(sb_))
                    if pt_b is not None:
                        ptcs[b] = stage_mask(pt_b, ins[b][2])

            if repeat == 1:
                body()
            else:
                with tc.For_i(0, repeat, 1):
                    body()

    nc.finalize()
    return nc


_NC_CACHE = {}


def _get_nc():
    if "nc" not in _NC_CACHE:
        _NC_CACHE["nc"] = build_nc()
    return _NC_CACHE["nc"]


def prep_in_maps(inputs, mask_array, dw1, dw2, dw3, db1, db2, db3):
    X = np.ascontiguousarray(
        np.asarray(inputs, dtype=np.float32).reshape(
            BATCH * BLOCK_NUM, BLOCK_LEN, DIM))
    m = np.asarray(mask_array, dtype=np.float32).reshape(
        BATCH * BLOCK_NUM, BLOCK_LEN, DIM)
    nb = BATCH * BLOCK_NUM

    # X^T variants --------------------------------------------------------
    xt = X.transpose(0, 2, 1)                      # [b, d, t]
    xt8 = (xt * np.float32(1.0 / C_X)).astype(NP_F8)
    # xt8int[b, p, i, tw, c, j] = xt8[b, 128*(2i+j)+p, tw*128 + (127-c)]
    tmp = xt8.reshape(nb, 2, 2, NC_P, NCH, NC_P)   # [b, i, j, p, tw, c]
    x8i = np.ascontiguousarray(
        tmp[:, :, :, :, :, ::-1].transpose(0, 3, 1, 4, 5, 2))
    # x8p[b, p, dc, t]: plain X^T/2 for the contiguous A-moving read
    x8p = np.ascontiguousarray(
        xt8.reshape(nb, NCH, NC_P, BLOCK_LEN).transpose(0, 2, 1, 3))
    # xtb[b, p, dc, k] = bf16 X[b, k, dc*128+p]
    xtb = np.ascontiguousarray(
        xt.reshape(nb, NCH, NC_P, BLOCK_LEN).transpose(0, 2, 1, 3)
    ).astype(NP_BF16)
    # xn[b, p, c, d] = X[b, c*128 + (127-p), d]: rows reversed per window
    # to match the swint-reversed O output partitions
    xn = np.ascontiguousarray(
        X.reshape(nb, NCH, NC_P, DIM)[:, :, ::-1, :].transpose(0, 2, 1, 3)
    ).astype(NP_BF16)
    # m8[b, p, kc, q] = m^T (q natural; pt free axis is natural now)
    mT = m.transpose(0, 2, 1)                      # [b, k, q]
    m8 = np.ascontiguousarray(
        mT.reshape(nb, NCH, NC_P, BLOCK_LEN).transpose(0, 2, 1, 3)
    ).astype(NP_F8)

    # weights -------------------------------------------------------------
    scale = np.float32(C_W12 / math.sqrt(DIM))
    w12 = ((np.asarray(dw1, np.float32) @ np.asarray(dw2, np.float32).T)
           * scale).astype(NP_F8)
    # w12int[p, i, d2w, c, j] = w12[128*(2i+j)+p, d2w*128 + (127-c)]
    t2 = np.asarray(w12).reshape(2, 2, NC_P, NCH, NC_P)  # [i, j, p, d2w, c]
    w12i = np.ascontiguousarray(
        t2[:, :, :, :, ::-1].transpose(2, 0, 3, 4, 1))
    w3 = np.ascontiguousarray(
        (np.asarray(dw3, np.float32) * np.float32(C_X))
        .reshape(NCH, NC_P, DIM).transpose(1, 0, 2)).astype(NP_F8)

    in_maps = []
    for c in range(N_CORES):
        s = slice(c * NBLK, (c + 1) * NBLK)
        in_maps.append({"x8i": x8i[s], "x8p": x8p[s], "xtb": xtb[s],
                        "xn": xn[s], "m8": m8[s], "w12i": w12i, "w3": w3})
    return in_maps


def kernel(inputs, mask_array, dw1, dw2, dw3, db1, db2, db3):
    nc = _get_nc()
    in_maps = prep_in_maps(inputs, mask_array, dw1, dw2, dw3, db1, db2, db3)
    res = run_bass_kernel_spmd(nc, in_maps, list(range(N_CORES)))
    out = np.concatenate(
        [np.asarray(res.results[c]["out"]) for c in range(N_CORES)], axis=0)
    # out[b, p, c, d] (f16), rows reversed per window -> [b, c*128+q, d]
    out = out.astype(np.float32)[:, ::-1, :, :].transpose(0, 2, 1, 3).reshape(
        BATCH, BLOCK_NUM, BLOCK_LEN, DIM)
    return np.ascontiguousarray(out)


# revision 21
# speedup vs baseline: 1.1487x; 1.1487x over previous
"""Block-local attention + LayerNorm kernel for Trainium2 (8 NeuronCores).

Problem (see reference):
  inputs [B=4, bn=16, bl=512, dim=512] fp32
  Q = X@W1, K = X@W2, V = X@W3 (+zero biases)
  S = Q K^T / sqrt(512), masked by elementwise {0,1} mask, softmax over keys
  out = LayerNorm(P @ V + X, eps=1e-3)

Sharding: 64 independent (batch, block) pairs -> 8 blocks per core.

v2 design, built from microbenchmarked instruction costs:
  * fp8 DoubleRow matmuls with ROTATING stationaries cost ~253ns (LDW
    serializes), but DoubleRowSwInterleave with host-interleaved weights
    streams at ~82ns.  A (=X W1W2^T) and V use swint stationaries
    (w12int, xt8int); numerics verified against numpy (ops_smoke.py).
  * S keeps bf16 (fp8 S busts the 2e-2 error gate; emulated 2.37e-2),
    16 rotating bf16 matmuls at ~140ns.
  * Row sums of P ride an augmented ones-column in the O matmul
    (N=256 + N=257 halves, 2-bank PSUM pair per q-chunk); no separate
    rowsum matmuls.  r lands in PSUM col 256 and feeds the z-stt scalar
    directly.
  * Mask is applied multiplicatively POST-exp (exp(S)*m == exp(S+log m)
    for m in {0,1}), as an fp8 multiply on the Pool engine that also
    performs the swint interleave-rearrange of pt into ptC via a 5-dim
    strided AP.  Mask ships as fp8 {0,1} (exact), halving its DMA.
  * Output ships as fp16 (emulated rel_err unchanged at 1.57e-2) and is
    upcast on the host.
  * Engine split per block: PE S+A+V+O ~4.5us; ACT exp+v+at/2+ob/2;
    DVE at/2+z+bn_stats+LN tail; Pool maskmul+memset (Pool cannot
    access PSUM - verified); SP all DMA.
  * Reversal bookkeeping: swint reverses stationary columns in HW.
    w12int/xt8int ship column-reversed so at/V come out normal.  The
    interleaved A-moving read makes at's free (q) axis reversed per
    128-window; m8 ships q-reversed to match; ptC (on-chip, cannot
    pre-reverse) then yields TRUE-normal q partitions in O because the
    two reversals cancel.  Everything downstream is natural.
"""

import math
import sys

import numpy as np

sys.path.insert(0, "/opt/trn_rl_repo")

import ml_dtypes

import concourse.bacc as bacc
import concourse.tile as tile
from concourse import mybir
from concourse.bass_utils import run_bass_kernel_spmd

DIM = 512
BLOCK_NUM = 16
SEQ_LEN = 8192
BLOCK_LEN = 512
BATCH = 4
LN_EPS = 1e-3
N_CORES = 8
NBLK = (BATCH * BLOCK_NUM) // N_CORES  # blocks per core
NC_P = 128
NCH = DIM // NC_P  # 4 chunks of 128

F32 = mybir.dt.float32
F16 = mybir.dt.float16
BF16 = mybir.dt.bfloat16
F8 = mybir.dt.float8e4
I32 = mybir.dt.int32
DRSW = mybir.MatmulPerfMode.DoubleRowSwInterleave
EXP = mybir.ActivationFunctionType.Exp
IDENT_FN = mybir.ActivationFunctionType.Identity
MUL = mybir.AluOpType.mult
ADD = mybir.AluOpType.add

NP_F8 = ml_dtypes.float8_e4m3
NP_BF16 = ml_dtypes.bfloat16

EXP_BIAS = -2.0   # headroom shift; cancels in the normalization
C_X = 2.0         # xt8 = X / 2
C_W12 = 256.0     # w12 = 256 * (W1 W2^T / sqrt(d)); A_psum = 128*A
S_SCALE = 1.0 / 128.0  # exp scale: S_psum = 128*S


def _form_a(ap3):
    """[p, c, j] contiguous 256B window -> swint lhsT [p, 2, 128]."""
    return ap3.rearrange("p c j -> p (c j)").rearrange(
        "p (a b) -> p a b", a=2)


ABLATE = set()  # timing diagnostics: {"ln","z","o","sexp","dma1","noswint"}


def build_nc(nblk=NBLK, repeat=1):
    ablate = ABLATE
    nc = bacc.Bacc("TRN2", target_bir_lowering=False, debug=False,
                   num_devices=N_CORES)

    x8i_d = nc.declare_dram_parameter(
        "x8i", [nblk, NC_P, 2, NCH, NC_P, 2], F8, isOutput=False)
    x8p_d = nc.declare_dram_parameter(
        "x8p", [nblk, NC_P, NCH, DIM], F8, isOutput=False)
    xtb_d = nc.declare_dram_parameter(
        "xtb", [nblk, NC_P, NCH, DIM], BF16, isOutput=False)
    m8_d = nc.declare_dram_parameter(
        "m8", [nblk, NC_P, NCH, DIM], F8, isOutput=False)
    xn_d = nc.declare_dram_parameter(
        "xn", [nblk, NC_P, NCH, DIM], BF16, isOutput=False)
    w12i_d = nc.declare_dram_parameter(
        "w12i", [NC_P, 2, NCH, NC_P, 2], F8, isOutput=False)
    w3_d = nc.declare_dram_parameter(
        "w3", [NC_P, NCH, DIM], F8, isOutput=False)
    out_d = nc.declare_dram_parameter(
        "out", [nblk, NC_P, NCH, DIM], F16, isOutput=True)

    with tile.TileContext(nc) as tc:
        with (
            tc.tile_pool(name="const", bufs=1) as const,
            tc.tile_pool(name="x8i", bufs=3) as p_x8i,
            tc.tile_pool(name="x8p", bufs=3) as p_x8p,
            tc.tile_pool(name="xtb", bufs=3) as p_xtb,
            tc.tile_pool(name="m8", bufs=3) as p_m8,
            tc.tile_pool(name="xn", bufs=5) as p_xn,
            tc.tile_pool(name="at", bufs=3) as p_at,
            tc.tile_pool(name="v8", bufs=4) as p_v8,
            tc.tile_pool(name="pt", bufs=3) as p_pt,
            tc.tile_pool(name="ptc", bufs=4) as p_ptc,
            tc.tile_pool(name="z", bufs=4) as p_z,
            tc.tile_pool(name="ob", bufs=2) as p_ob,
            tc.tile_pool(name="tiny", bufs=4) as p_tiny,
            tc.tile_pool(name="wave", bufs=4, space="PSUM") as ps_wave,
        ):
            w12i = const.tile([NC_P, 2, NCH, NC_P, 2], F8)
            nc.sync.dma_start(out=w12i, in_=w12i_d[:])
            w3 = const.tile([NC_P, NCH, DIM], F8)
            nc.sync.dma_start(out=w3, in_=w3_d[:])
            ebias = const.tile([NC_P, 1], F32)
            nc.vector.memset(ebias, EXP_BIAS)

            def load_inputs(b):
                x8i = p_x8i.tile([NC_P, 2, NCH, NC_P, 2], F8, tag="x8i",
                                 name="x8i_sb")
                x8p = p_x8p.tile([NC_P, NCH, DIM], F8, tag="x8p",
                                 name="x8p_sb")
                xtb = p_xtb.tile([NC_P, NCH, DIM], BF16, tag="xtb",
                                 name="xtb_sb")
                m8 = p_m8.tile([NC_P, NCH, DIM], F8, tag="m8", name="m8_sb")
                xn = p_xn.tile([NC_P, NCH, DIM], BF16, tag="xn", name="xn_sb")
                if "dma1" in ablate and b > 0:
                    nc.sync.dma_start(out=x8i[:, 0], in_=x8i_d[b, :, 0])
                    nc.sync.dma_start(out=x8p[:, 0:1, :],
                                      in_=x8p_d[b, :, 0:1])
                    nc.sync.dma_start(out=xtb[:, 0:1, :], in_=xtb_d[b, :, 0:1])
                    nc.sync.dma_start(out=m8[:, 0:1, :], in_=m8_d[b, :, 0:1])
                    nc.sync.dma_start(out=xn[:, 0:1, :], in_=xn_d[b, :, 0:1])
                else:
                    nc.sync.dma_start(out=x8i, in_=x8i_d[b])
                    nc.sync.dma_start(out=x8p, in_=x8p_d[b])
                    nc.sync.dma_start(out=xtb, in_=xtb_d[b])
                    nc.sync.dma_start(out=m8, in_=m8_d[b])
                    nc.sync.dma_start(out=xn, in_=xn_d[b])
                return (x8i, x8p), xtb, m8, xn

            def stage_a(x8pair):
                x8i, x8p = x8pair
                """A^T = w12int^T (X^T/2): 8 swint MMs, out at bf16(128*A)."""
                at = p_at.tile([NC_P, NCH, DIM], BF16, tag="at", name="at_sb")
                for w in range(2):
                    ps = ps_wave.tile([NC_P, 2, DIM], F32, tag="wave",
                                      name="psA")
                    for h in range(2):
                        d2c = 2 * w + h
                        for i in range(2):
                            nc.tensor.matmul(
                                ps[:, h, :],
                                lhsT=_form_a(w12i[:, i, d2c]),
                                rhs=x8p[:, 2 * i:2 * i + 2, :],
                                start=(i == 0), stop=(i == 1),
                                perf_mode=(mybir.MatmulPerfMode.DoubleRow
                                           if "noswint" in ablate else DRSW))
                    nc.scalar.copy(at[:, 2 * w:2 * w + 2, :], ps[:])
                return at

            def stage_s(b, xtb, at, m8):
                """S^T = X^T^T A^T (bf16), exp -> pt."""
                pt = p_pt.tile([NC_P, NCH, DIM], F8, tag="pt", name="pt_sb")
                for w in range(2):
                    ps = ps_wave.tile([NC_P, 2, DIM], F32, tag="wave",
                                      name="psS")
                    for h in range(2):
                        kc = 2 * w + h
                        for dc in range(NCH):
                            nc.tensor.matmul(
                                ps[:, h, :],
                                lhsT=xtb[:, dc, kc * NC_P:(kc + 1) * NC_P],
                                rhs=at[:, dc, :],
                                start=(dc == 0), stop=(dc == NCH - 1))
                    if "sexp" in ablate:
                        nc.scalar.copy(pt[:, 2 * w:2 * w + 2, :], ps[:])
                    else:
                        nc.scalar.activation(pt[:, 2 * w:2 * w + 2, :],
                                             ps[:], EXP, bias=ebias[:],
                                             scale=S_SCALE)
                return pt

            def stage_mask(pt, m8):
                """ptC = (pt * mask) interleave-rearranged, one fused
                DVE op (measured 1.69us; Pool is ~2x slower/elem and ACT
                cannot do tensor*tensor)."""
                ptc = p_ptc.tile([NC_P, 2, NCH, NC_P, 2], F8, tag="ptc",
                                 name="ptc_sb")
                nc.vector.tensor_mul(
                    ptc[:].transpose([0, 1, 4, 2, 3]),
                    pt[:].rearrange("p (i j) (qc c) -> p i j qc c",
                                    i=2, j=2, qc=NCH, c=NC_P),
                    m8[:].rearrange("p (i j) (qc c) -> p i j qc c",
                                    i=2, j=2, qc=NCH, c=NC_P))
                return ptc

            def stage_v(x8pair):
                x8i, _ = x8pair
                """V = (X^T/2)^T (2 W3): 8 swint MMs -> v8 fp8 + ones col."""
                v8 = p_v8.tile([NC_P, NCH, 516], F8, tag="v8", name="v8_sb")
                for w in range(2):
                    ps = ps_wave.tile([NC_P, 2, DIM], F32, tag="wave",
                                      name="psV")
                    for h in range(2):
                        tc_i = 2 * w + h
                        for i in range(2):
                            nc.tensor.matmul(
                                ps[:, h, :],
                                lhsT=_form_a(x8i[:, i, tc_i]),
                                rhs=w3[:, 2 * i:2 * i + 2, :],
                                start=(i == 0), stop=(i == 1),
                                perf_mode=DRSW)
                    nc.scalar.copy(v8[:, 2 * w:2 * w + 2, 0:DIM], ps[:])
                nc.gpsimd.memset(v8[:, :, 512:513], 1.0)
                return v8

            def stage_o(ptc, v8, xn):
                """O = P_u^T^T [V|1]: 16 aug swint MMs; z = r*xn + O."""
                z = p_z.tile([NC_P, NCH, DIM], BF16, tag="z", name="z_sb")
                r_sb = p_tiny.tile([NC_P, NCH], F32, tag="r", name="r_sb")
                for qc in range(NCH):
                    ps = ps_wave.tile([NC_P, 2, DIM], F32, tag="wave",
                                      name="psO")
                    for i in range(2):
                        lhs = _form_a(ptc[:, i, qc])
                        nc.tensor.matmul(
                            ps[:, 0, 0:256], lhsT=lhs,
                            rhs=v8[:, 2 * i:2 * i + 2, 0:256],
                            start=(i == 0), stop=(i == 1), perf_mode=DRSW)
                        nc.tensor.matmul(
                            ps[:, 1, 0:257], lhsT=lhs,
                            rhs=v8[:, 2 * i:2 * i + 2, 256:513],
                            start=(i == 0), stop=(i == 1), perf_mode=DRSW)
                    if "z" not in ablate:
                        nc.vector.scalar_tensor_tensor(
                            out=z[:, qc, :].rearrange("p (a c) -> p a c",
                                                      a=2),
                            in0=xn[:, qc, :].rearrange("p (a c) -> p a c",
                                                       a=2),
                            scalar=ps[:, 1, 256:257],
                            in1=ps[:, :, 0:256],
                            op0=MUL, op1=ADD)
                        nc.vector.tensor_copy(r_sb[:, qc:qc + 1],
                                              ps[:, 1, 256:257])
                return z, r_sb

            def stage_stats(z, r_sb):
                """bn stats + istd = rsqrt(var + eps*r^2) via magic Newton.

                Small scalar chain runs on Pool (SBUF-only engine, idle)."""
                mvb = p_tiny.tile([NC_P, NCH, 2], F32, tag="mvb", name="mvb")
                for qc in range(NCH):
                    stats = p_tiny.tile([NC_P, 6], F32, tag="st", name="st")
                    nc.vector.bn_stats(stats[:], z[:, qc, :])
                    nc.vector.bn_aggr(mvb[:, qc, :], stats[:])
                rr = p_tiny.tile([NC_P, NCH], F32, tag="rr", name="rr")
                nc.gpsimd.tensor_mul(rr[:], r_sb[:], r_sb[:])
                nc.gpsimd.tensor_scalar_mul(rr[:], rr[:], LN_EPS)
                tv = p_tiny.tile([NC_P, NCH], F32, tag="tv", name="tv")
                nc.gpsimd.tensor_add(tv[:], rr[:], mvb[:, :, 1])
                yv = p_tiny.tile([NC_P, NCH], F32, tag="yv", name="yv")
                hv = p_tiny.tile([NC_P, NCH], F32, tag="hv", name="hv")
                nc.vector.tensor_scalar(
                    out=hv[:].bitcast(I32), in0=tv[:].bitcast(I32),
                    scalar1=1, scalar2=None,
                    op0=mybir.AluOpType.logical_shift_right)
                nc.vector.tensor_scalar(
                    out=yv[:].bitcast(I32), in0=hv[:].bitcast(I32),
                    scalar1=-1, scalar2=0x5F3759DF,
                    op0=MUL, op1=ADD)
                av = p_tiny.tile([NC_P, NCH], F32, tag="av", name="av")
                cv = p_tiny.tile([NC_P, NCH], F32, tag="cv", name="cv")
                for _ in range(2):
                    nc.gpsimd.tensor_mul(av[:], yv[:], yv[:])
                    nc.gpsimd.tensor_mul(av[:], av[:], tv[:])
                    nc.vector.tensor_scalar(
                        out=cv[:], in0=av[:], scalar1=-0.5, scalar2=1.5,
                        op0=MUL, op1=ADD)
                    nc.gpsimd.tensor_mul(yv[:], yv[:], cv[:])
                negms = p_tiny.tile([NC_P, NCH], F32, tag="negms",
                                    name="negms")
                nc.gpsimd.tensor_mul(negms[:], mvb[:, :, 0], yv[:])
                nc.gpsimd.tensor_scalar_mul(negms[:], negms[:], -1.0)
                return yv, negms

            def stage_ob(b, z, yv, negms):
                """Apply LN affine, f16 out, ship."""
                ob = p_ob.tile([NC_P, NCH, DIM], F16, tag="ob", name="ob_sb")
                for qc in range(NCH):
                    if qc < 3:
                        nc.scalar.activation(
                            ob[:, qc, :], z[:, qc, :], IDENT_FN,
                            bias=negms[:, qc:qc + 1],
                            scale=yv[:, qc:qc + 1])
                    else:
                        nc.vector.tensor_scalar(
                            out=ob[:, qc, :], in0=z[:, qc, :],
                            scalar1=yv[:, qc:qc + 1],
                            scalar2=negms[:, qc:qc + 1],
                            op0=MUL, op1=ADD)
                nc.sync.dma_start(out=out_d[b], in_=ob[:])

            def body():
                # deep software pipeline: per iteration b,
                #   ob(b-4) | S(b), A(b+1), V(b) | O(b-2) | stats(b-3)
                # so every op's producers are >=1 iteration old, and the
                # mask->interleave chain gets ~1.5 iterations of slack
                # before O consumes ptC.
                ins = {}
                ats = {}
                ptcs = {}
                v8s = {}
                zs = {}
                rs = {}
                lns = {}
                ins[0] = load_inputs(0)
                if "noav" not in ablate:
                    ats[0] = stage_a(ins[0][0])
                for b in range(nblk + 4):
                    if b >= 4 and "ln" not in ablate and "o" not in ablate \
                            and "z" not in ablate:
                        zb = zs.pop(b - 4)
                        yvb, ngb = lns.pop(b - 4)
                        stage_ob(b - 4, zb, yvb, ngb)
                    pt_b = None
                    if b < nblk:
                        x8pr, xtb, m8, xn = ins[b]
                        mov = xtb if "noav" in ablate else ats.pop(b)
                        pt_b = stage_s(b, xtb, mov, m8)
                        if b + 1 < nblk:
                            ins[b + 1] = load_inputs(b + 1)
                            if "noav" not in ablate:
                                ats[b + 1] = stage_a(ins[b + 1][0])
                        if "noav" not in ablate and "nov" not in ablate:
                            v8s[b] = stage_v(x8pr)
                    if 2 <= b <= nblk + 1 and "o" not in ablate:
                        pb = b - 2
                        x8i_p, xtb_p, m8_p, xn_p = ins.pop(pb)  # noqa
                        zs[pb], rs[pb] = stage_o(ptcs.pop(pb), v8s.pop(pb),
                                                 xn_p)
                    if 3 <= b <= nblk + 2 and "ln" not in ablate \
                            and "o" not in ablate and "z" not in ablate:
                        sb_ = b - 3
                        lns[sb_] = stage_stats(zs[sb_], rs.pop(sb_))
                    if pt_b is not None:
                        ptcs[b] = stage_mask(pt_b, ins[b][2])

            if repeat == 1:
                body()
            else:
                with tc.For_i(0, repeat, 1):
                    body()

    nc.finalize()
    return nc


_NC_CACHE = {}


def _get_nc():
    if "nc" not in _NC_CACHE:
        _NC_CACHE["nc"] = build_nc()
    return _NC_CACHE["nc"]


def prep_in_maps(inputs, mask_array, dw1, dw2, dw3, db1, db2, db3):
    X = np.ascontiguousarray(
        np.asarray(inputs, dtype=np.float32).reshape(
            BATCH * BLOCK_NUM, BLOCK_LEN, DIM))
    m = np.asarray(mask_array, dtype=np.float32).reshape(
        BATCH * BLOCK_NUM, BLOCK_LEN, DIM)
    nb = BATCH * BLOCK_NUM

    # X^T variants --------------------------------------------------------
    xt = X.transpose(0, 2, 1)                      # [b, d, t]
    xt8 = (xt * np.float32(1.0 / C_X)).astype(NP_F8)
    # xt8int[b, p, i, tw, c, j] = xt8[b, 128*(2i+j)+p, tw*128 + (127-c)]
    tmp = xt8.reshape(nb, 2, 2, NC_P, NCH, NC_P)   # [b, i, j, p, tw, c]
    x8i = np.ascontiguousarray(
        tmp[:, :, :, :, :, ::-1].transpose(0, 3, 1, 4, 5, 2))
    # x8p[b, p, dc, t]: plain X^T/2 for the contiguous A-moving read
    x8p = np.ascontiguousarray(
        xt8.reshape(nb, NCH, NC_P, BLOCK_LEN).transpose(0, 2, 1, 3))
    # xtb[b, p, dc, k] = bf16 X[b, k, dc*128+p]
    xtb = np.ascontiguousarray(
        xt.reshape(nb, NCH, NC_P, BLOCK_LEN).transpose(0, 2, 1, 3)
    ).astype(NP_BF16)
    # xn[b, p, c, d] = X[b, c*128 + (127-p), d]: rows reversed per window
    # to match the swint-reversed O output partitions
    xn = np.ascontiguousarray(
        X.reshape(nb, NCH, NC_P, DIM)[:, :, ::-1, :].transpose(0, 2, 1, 3)
    ).astype(NP_BF16)
    # m8[b, p, kc, q] = m^T (q natural; pt free axis is natural now)
    mT = m.transpose(0, 2, 1)                      # [b, k, q]
    m8 = np.ascontiguousarray(
        mT.reshape(nb, NCH, NC_P, BLOCK_LEN).transpose(0, 2, 1, 3)
    ).astype(NP_F8)

    # weights -------------------------------------------------------------
    scale = np.float32(C_W12 / math.sqrt(DIM))
    w12 = ((np.asarray(dw1, np.float32) @ np.asarray(dw2, np.float32).T)
           * scale).astype(NP_F8)
    # w12int[p, i, d2w, c, j] = w12[128*(2i+j)+p, d2w*128 + (127-c)]
    t2 = np.asarray(w12).reshape(2, 2, NC_P, NCH, NC_P)  # [i, j, p, d2w, c]
    w12i = np.ascontiguousarray(
        t2[:, :, :, :, ::-1].transpose(2, 0, 3, 4, 1))
    w3 = np.ascontiguousarray(
        (np.asarray(dw3, np.float32) * np.float32(C_X))
        .reshape(NCH, NC_P, DIM).transpose(1, 0, 2)).astype(NP_F8)

    in_maps = []
    for c in range(N_CORES):
        s = slice(c * NBLK, (c + 1) * NBLK)
        in_maps.append({"x8i": x8i[s], "x8p": x8p[s], "xtb": xtb[s],
                        "xn": xn[s], "m8": m8[s], "w12i": w12i, "w3": w3})
    return in_maps


def kernel(inputs, mask_array, dw1, dw2, dw3, db1, db2, db3):
    nc = _get_nc()
    in_maps = prep_in_maps(inputs, mask_array, dw1, dw2, dw3, db1, db2, db3)
    res = run_bass_kernel_spmd(nc, in_maps, list(range(N_CORES)))
    out = np.concatenate(
        [np.asarray(res.results[c]["out"]) for c in range(N_CORES)], axis=0)
    # out[b, p, c, d] (f16), rows reversed per window -> [b, c*128+q, d]
    out = out.astype(np.float32)[:, ::-1, :, :].transpose(0, 2, 1, 3).reshape(
        BATCH, BLOCK_NUM, BLOCK_LEN, DIM)
    return np.ascontiguousarray(out)


# revision 24
# speedup vs baseline: 1.2210x; 1.0629x over previous
"""Block-local attention + LayerNorm kernel for Trainium2 (8 NeuronCores).

Problem (see reference):
  inputs [B=4, bn=16, bl=512, dim=512] fp32
  Q = X@W1, K = X@W2, V = X@W3 (+zero biases)
  S = Q K^T / sqrt(512), masked by elementwise {0,1} mask, softmax over keys
  out = LayerNorm(P @ V + X, eps=1e-3)

Sharding: 64 independent (batch, block) pairs -> 8 blocks per core.

v2 design, built from microbenchmarked instruction costs:
  * fp8 DoubleRow matmuls with ROTATING stationaries cost ~253ns (LDW
    serializes), but DoubleRowSwInterleave with host-interleaved weights
    streams at ~82ns.  A (=X W1W2^T) and V use swint stationaries
    (w12int, xt8int); numerics verified against numpy (ops_smoke.py).
  * S keeps bf16 (fp8 S busts the 2e-2 error gate; emulated 2.37e-2),
    16 rotating bf16 matmuls at ~140ns.
  * Row sums of P ride an augmented ones-column in the O matmul
    (N=256 + N=257 halves, 2-bank PSUM pair per q-chunk); no separate
    rowsum matmuls.  r lands in PSUM col 256 and feeds the z-stt scalar
    directly.
  * Mask is applied multiplicatively POST-exp (exp(S)*m == exp(S+log m)
    for m in {0,1}), as an fp8 multiply on the Pool engine that also
    performs the swint interleave-rearrange of pt into ptC via a 5-dim
    strided AP.  Mask ships as fp8 {0,1} (exact), halving its DMA.
  * Output ships as fp16 (emulated rel_err unchanged at 1.57e-2) and is
    upcast on the host.
  * Engine split per block: PE S+A+V+O ~4.5us; ACT exp+v+at/2+ob/2;
    DVE at/2+z+bn_stats+LN tail; Pool maskmul+memset (Pool cannot
    access PSUM - verified); SP all DMA.
  * Reversal bookkeeping: swint reverses stationary columns in HW.
    w12int/xt8int ship column-reversed so at/V come out normal.  The
    interleaved A-moving read makes at's free (q) axis reversed per
    128-window; m8 ships q-reversed to match; ptC (on-chip, cannot
    pre-reverse) then yields TRUE-normal q partitions in O because the
    two reversals cancel.  Everything downstream is natural.
"""

import math
import sys

import numpy as np

sys.path.insert(0, "/opt/trn_rl_repo")

import ml_dtypes

import concourse.bacc as bacc
import concourse.tile as tile
from concourse import mybir
from concourse.bass_utils import run_bass_kernel_spmd

DIM = 512
BLOCK_NUM = 16
SEQ_LEN = 8192
BLOCK_LEN = 512
BATCH = 4
LN_EPS = 1e-3
N_CORES = 8
NBLK = (BATCH * BLOCK_NUM) // N_CORES  # blocks per core
NC_P = 128
NCH = DIM // NC_P  # 4 chunks of 128

F32 = mybir.dt.float32
F16 = mybir.dt.float16
BF16 = mybir.dt.bfloat16
F8 = mybir.dt.float8e4
I32 = mybir.dt.int32
DRSW = mybir.MatmulPerfMode.DoubleRowSwInterleave
EXP = mybir.ActivationFunctionType.Exp
IDENT_FN = mybir.ActivationFunctionType.Identity
MUL = mybir.AluOpType.mult
ADD = mybir.AluOpType.add

NP_F8 = ml_dtypes.float8_e4m3
NP_BF16 = ml_dtypes.bfloat16

EXP_BIAS = -2.0   # headroom shift; cancels in the normalization
C_X = 2.0         # xt8 = X / 2
C_W12 = 256.0     # w12 = 256 * (W1 W2^T / sqrt(d)); A_psum = 128*A
S_SCALE = 1.0 / 128.0  # exp scale: S_psum = 128*S


def _form_a(ap3):
    """[p, c, j] contiguous 256B window -> swint lhsT [p, 2, 128]."""
    return ap3.rearrange("p c j -> p (c j)").rearrange(
        "p (a b) -> p a b", a=2)


ABLATE = set()  # timing diagnostics: {"ln","z","o","sexp","dma1","noswint"}
USE_X8P = True   # ship plain xt8 for contiguous A-moving reads
LAG = 1          # O-stage lag in blocks (1 or 2)
MASK_EARLY = False  # issue maskmul right after exp instead of end-of-iter


def build_nc(nblk=NBLK, repeat=1):
    ablate = ABLATE
    nc = bacc.Bacc("TRN2", target_bir_lowering=False, debug=False,
                   num_devices=N_CORES)

    x8i_d = nc.declare_dram_parameter(
        "x8i", [nblk, NC_P, 2, NCH, NC_P, 2], F8, isOutput=False)
    x8p_d = (nc.declare_dram_parameter(
        "x8p", [nblk, NC_P, NCH, DIM], F8, isOutput=False)
        if USE_X8P else None)
    xtb_d = nc.declare_dram_parameter(
        "xtb", [nblk, NC_P, NCH, DIM], BF16, isOutput=False)
    m8_d = nc.declare_dram_parameter(
        "m8", [nblk, NC_P, NCH, DIM], F8, isOutput=False)
    xn_d = nc.declare_dram_parameter(
        "xn", [nblk, NC_P, NCH, DIM], BF16, isOutput=False)
    w12i_d = nc.declare_dram_parameter(
        "w12i", [NC_P, 2, NCH, NC_P, 2], F8, isOutput=False)
    w3_d = nc.declare_dram_parameter(
        "w3", [NC_P, NCH, DIM], F8, isOutput=False)
    out_d = nc.declare_dram_parameter(
        "out", [nblk, NC_P, NCH, DIM], F16, isOutput=True)

    with tile.TileContext(nc) as tc:
        with (
            tc.tile_pool(name="const", bufs=1) as const,
            tc.tile_pool(name="x8i", bufs=3) as p_x8i,
            tc.tile_pool(name="x8p", bufs=3) as p_x8p,
            tc.tile_pool(name="xtb", bufs=3) as p_xtb,
            tc.tile_pool(name="m8", bufs=3) as p_m8,
            tc.tile_pool(name="xn", bufs=5) as p_xn,
            tc.tile_pool(name="at", bufs=3) as p_at,
            tc.tile_pool(name="v8", bufs=4) as p_v8,
            tc.tile_pool(name="pt", bufs=3) as p_pt,
            tc.tile_pool(name="ptc", bufs=4) as p_ptc,
            tc.tile_pool(name="z", bufs=4) as p_z,
            tc.tile_pool(name="ob", bufs=2) as p_ob,
            tc.tile_pool(name="tiny", bufs=4) as p_tiny,
            tc.tile_pool(name="wave", bufs=4, space="PSUM") as ps_wave,
        ):
            w12i = const.tile([NC_P, 2, NCH, NC_P, 2], F8)
            nc.sync.dma_start(out=w12i, in_=w12i_d[:])
            w3 = const.tile([NC_P, NCH, DIM], F8)
            nc.sync.dma_start(out=w3, in_=w3_d[:])
            ebias = const.tile([NC_P, 1], F32)
            nc.vector.memset(ebias, EXP_BIAS)

            def load_inputs(b):
                x8i = p_x8i.tile([NC_P, 2, NCH, NC_P, 2], F8, tag="x8i",
                                 name="x8i_sb")
                x8p = (p_x8p.tile([NC_P, NCH, DIM], F8, tag="x8p",
                                  name="x8p_sb") if USE_X8P else None)
                xtb = p_xtb.tile([NC_P, NCH, DIM], BF16, tag="xtb",
                                 name="xtb_sb")
                m8 = p_m8.tile([NC_P, NCH, DIM], F8, tag="m8", name="m8_sb")
                xn = p_xn.tile([NC_P, NCH, DIM], BF16, tag="xn", name="xn_sb")
                if "dma1" in ablate and b > 0:
                    nc.sync.dma_start(out=x8i[:, 0], in_=x8i_d[b, :, 0])
                    if USE_X8P:
                        nc.sync.dma_start(out=x8p[:, 0:1, :],
                                          in_=x8p_d[b, :, 0:1])
                    nc.sync.dma_start(out=xtb[:, 0:1, :], in_=xtb_d[b, :, 0:1])
                    nc.sync.dma_start(out=m8[:, 0:1, :], in_=m8_d[b, :, 0:1])
                    nc.sync.dma_start(out=xn[:, 0:1, :], in_=xn_d[b, :, 0:1])
                else:
                    nc.sync.dma_start(out=x8i, in_=x8i_d[b])
                    if USE_X8P:
                        nc.sync.dma_start(out=x8p, in_=x8p_d[b])
                    nc.sync.dma_start(out=xtb, in_=xtb_d[b])
                    nc.sync.dma_start(out=m8, in_=m8_d[b])
                    nc.sync.dma_start(out=xn, in_=xn_d[b])
                return (x8i, x8p), xtb, m8, xn

            def stage_a(x8pair):
                x8i, x8p = x8pair
                """A^T = w12int^T (X^T/2): 8 swint MMs, out at bf16(128*A)."""
                at = p_at.tile([NC_P, NCH, DIM], BF16, tag="at", name="at_sb")
                for w in range(2):
                    ps = ps_wave.tile([NC_P, 2, DIM], F32, tag="wave",
                                      name="psA")
                    for h in range(2):
                        d2c = 2 * w + h
                        for i in range(2):
                            nc.tensor.matmul(
                                ps[:, h, :],
                                lhsT=_form_a(w12i[:, i, d2c]),
                                rhs=(x8p[:, 2 * i:2 * i + 2, :]
                                     if USE_X8P else
                                     x8i[:, i].transpose([0, 3, 1, 2])),
                                start=(i == 0), stop=(i == 1),
                                perf_mode=(mybir.MatmulPerfMode.DoubleRow
                                           if "noswint" in ablate else DRSW))
                    nc.scalar.copy(at[:, 2 * w:2 * w + 2, :], ps[:])
                return at

            def stage_s(b, xtb, at, m8):
                """S^T = X^T^T A^T (bf16), exp -> pt."""
                pt = p_pt.tile([NC_P, NCH, DIM], F8, tag="pt", name="pt_sb")
                for w in range(2):
                    ps = ps_wave.tile([NC_P, 2, DIM], F32, tag="wave",
                                      name="psS")
                    for h in range(2):
                        kc = 2 * w + h
                        for dc in range(NCH):
                            nc.tensor.matmul(
                                ps[:, h, :],
                                lhsT=xtb[:, dc, kc * NC_P:(kc + 1) * NC_P],
                                rhs=at[:, dc, :],
                                start=(dc == 0), stop=(dc == NCH - 1))
                    if "sexp" in ablate:
                        nc.scalar.copy(pt[:, 2 * w:2 * w + 2, :], ps[:])
                    else:
                        nc.scalar.activation(pt[:, 2 * w:2 * w + 2, :],
                                             ps[:], EXP, bias=ebias[:],
                                             scale=S_SCALE)
                return pt

            def stage_mask(pt, m8):
                """ptC = (pt * mask) interleave-rearranged, one fused
                DVE op (measured 1.69us; Pool is ~2x slower/elem and ACT
                cannot do tensor*tensor)."""
                ptc = p_ptc.tile([NC_P, 2, NCH, NC_P, 2], F8, tag="ptc",
                                 name="ptc_sb")
                nc.vector.tensor_mul(
                    ptc[:].transpose([0, 1, 4, 2, 3]),
                    pt[:].rearrange("p (i j) (qc c) -> p i j qc c",
                                    i=2, j=2, qc=NCH, c=NC_P),
                    m8[:].rearrange("p (i j) (qc c) -> p i j qc c",
                                    i=2, j=2, qc=NCH, c=NC_P))
                return ptc

            def stage_v(x8pair):
                x8i, _ = x8pair
                """V = (X^T/2)^T (2 W3): 8 swint MMs -> v8 fp8 + ones col."""
                v8 = p_v8.tile([NC_P, NCH, 516], F8, tag="v8", name="v8_sb")
                for w in range(2):
                    ps = ps_wave.tile([NC_P, 2, DIM], F32, tag="wave",
                                      name="psV")
                    for h in range(2):
                        tc_i = 2 * w + h
                        for i in range(2):
                            nc.tensor.matmul(
                                ps[:, h, :],
                                lhsT=_form_a(x8i[:, i, tc_i]),
                                rhs=w3[:, 2 * i:2 * i + 2, :],
                                start=(i == 0), stop=(i == 1),
                                perf_mode=DRSW)
                    nc.scalar.copy(v8[:, 2 * w:2 * w + 2, 0:DIM], ps[:])
                nc.gpsimd.memset(v8[:, :, 512:513], 1.0)
                return v8

            def stage_o(ptc, v8, xn):
                """O = P_u^T^T [V|1]: 16 aug swint MMs; z = r*xn + O."""
                z = p_z.tile([NC_P, NCH, DIM], BF16, tag="z", name="z_sb")
                r_sb = p_tiny.tile([NC_P, NCH], F32, tag="r", name="r_sb")
                for qc in range(NCH):
                    ps = ps_wave.tile([NC_P, 2, DIM], F32, tag="wave",
                                      name="psO")
                    for i in range(2):
                        lhs = _form_a(ptc[:, i, qc])
                        nc.tensor.matmul(
                            ps[:, 0, 0:256], lhsT=lhs,
                            rhs=v8[:, 2 * i:2 * i + 2, 0:256],
                            start=(i == 0), stop=(i == 1), perf_mode=DRSW)
                        nc.tensor.matmul(
                            ps[:, 1, 0:257], lhsT=lhs,
                            rhs=v8[:, 2 * i:2 * i + 2, 256:513],
                            start=(i == 0), stop=(i == 1), perf_mode=DRSW)
                    if "z" not in ablate:
                        nc.vector.scalar_tensor_tensor(
                            out=z[:, qc, :].rearrange("p (a c) -> p a c",
                                                      a=2),
                            in0=xn[:, qc, :].rearrange("p (a c) -> p a c",
                                                       a=2),
                            scalar=ps[:, 1, 256:257],
                            in1=ps[:, :, 0:256],
                            op0=MUL, op1=ADD)
                        nc.vector.tensor_copy(r_sb[:, qc:qc + 1],
                                              ps[:, 1, 256:257])
                return z, r_sb

            def stage_stats(z, r_sb):
                """bn stats + istd = rsqrt(var + eps*r^2) via magic Newton.

                Small scalar chain runs on Pool (SBUF-only engine, idle)."""
                mvb = p_tiny.tile([NC_P, NCH, 2], F32, tag="mvb", name="mvb")
                for qc in range(NCH):
                    stats = p_tiny.tile([NC_P, 6], F32, tag="st", name="st")
                    nc.vector.bn_stats(stats[:], z[:, qc, :])
                    nc.vector.bn_aggr(mvb[:, qc, :], stats[:])
                rr = p_tiny.tile([NC_P, NCH], F32, tag="rr", name="rr")
                nc.gpsimd.tensor_mul(rr[:], r_sb[:], r_sb[:])
                nc.gpsimd.tensor_scalar_mul(rr[:], rr[:], LN_EPS)
                tv = p_tiny.tile([NC_P, NCH], F32, tag="tv", name="tv")
                nc.gpsimd.tensor_add(tv[:], rr[:], mvb[:, :, 1])
                yv = p_tiny.tile([NC_P, NCH], F32, tag="yv", name="yv")
                hv = p_tiny.tile([NC_P, NCH], F32, tag="hv", name="hv")
                nc.vector.tensor_scalar(
                    out=hv[:].bitcast(I32), in0=tv[:].bitcast(I32),
                    scalar1=1, scalar2=None,
                    op0=mybir.AluOpType.logical_shift_right)
                nc.vector.tensor_scalar(
                    out=yv[:].bitcast(I32), in0=hv[:].bitcast(I32),
                    scalar1=-1, scalar2=0x5F3759DF,
                    op0=MUL, op1=ADD)
                av = p_tiny.tile([NC_P, NCH], F32, tag="av", name="av")
                cv = p_tiny.tile([NC_P, NCH], F32, tag="cv", name="cv")
                for _ in range(2):
                    nc.gpsimd.tensor_mul(av[:], yv[:], yv[:])
                    nc.gpsimd.tensor_mul(av[:], av[:], tv[:])
                    nc.vector.tensor_scalar(
                        out=cv[:], in0=av[:], scalar1=-0.5, scalar2=1.5,
                        op0=MUL, op1=ADD)
                    nc.gpsimd.tensor_mul(yv[:], yv[:], cv[:])
                negms = p_tiny.tile([NC_P, NCH], F32, tag="negms",
                                    name="negms")
                nc.gpsimd.tensor_mul(negms[:], mvb[:, :, 0], yv[:])
                nc.gpsimd.tensor_scalar_mul(negms[:], negms[:], -1.0)
                return yv, negms

            def stage_ob(b, z, yv, negms):
                """Apply LN affine, f16 out, ship."""
                ob = p_ob.tile([NC_P, NCH, DIM], F16, tag="ob", name="ob_sb")
                for qc in range(NCH):
                    if qc < 3:
                        nc.scalar.activation(
                            ob[:, qc, :], z[:, qc, :], IDENT_FN,
                            bias=negms[:, qc:qc + 1],
                            scale=yv[:, qc:qc + 1])
                    else:
                        nc.vector.tensor_scalar(
                            out=ob[:, qc, :], in0=z[:, qc, :],
                            scalar1=yv[:, qc:qc + 1],
                            scalar2=negms[:, qc:qc + 1],
                            op0=MUL, op1=ADD)
                nc.sync.dma_start(out=out_d[b], in_=ob[:])

            def body():
                # deep software pipeline: per iteration b,
                #   ob(b-4) | S(b), A(b+1), V(b) | O(b-2) | stats(b-3)
                # so every op's producers are >=1 iteration old, and the
                # mask->interleave chain gets ~1.5 iterations of slack
                # before O consumes ptC.
                ins = {}
                ats = {}
                ptcs = {}
                v8s = {}
                zs = {}
                rs = {}
                lns = {}
                ins[0] = load_inputs(0)
                if "noav" not in ablate:
                    ats[0] = stage_a(ins[0][0])
                LG = LAG
                for b in range(nblk + LG + 2):
                    if b >= LG + 2 and "ln" not in ablate \
                            and "o" not in ablate and "z" not in ablate:
                        zb = zs.pop(b - LG - 2)
                        yvb, ngb = lns.pop(b - LG - 2)
                        stage_ob(b - LG - 2, zb, yvb, ngb)
                    # O first on PE: its inputs are LG iterations old,
                    # so DVE's z-stt gets data immediately at iter start.
                    if LG <= b <= nblk + LG - 1 and "o" not in ablate:
                        pb = b - LG
                        x8i_p, xtb_p, m8_p, xn_p = ins.pop(pb)  # noqa
                        zs[pb], rs[pb] = stage_o(ptcs.pop(pb), v8s.pop(pb),
                                                 xn_p)
                    pt_b = None
                    if b < nblk:
                        x8pr, xtb, m8, xn = ins[b]
                        mov = xtb if "noav" in ablate else ats.pop(b)
                        pt_b = stage_s(b, xtb, mov, m8)
                        if MASK_EARLY:
                            ptcs[b] = stage_mask(pt_b, m8)
                            pt_b = None
                        if b + 1 < nblk:
                            ins[b + 1] = load_inputs(b + 1)
                            if "noav" not in ablate:
                                ats[b + 1] = stage_a(ins[b + 1][0])
                        if "noav" not in ablate and "nov" not in ablate:
                            v8s[b] = stage_v(x8pr)
                    if LG + 1 <= b <= nblk + LG and "ln" not in ablate \
                            and "o" not in ablate and "z" not in ablate:
                        sb_ = b - LG - 1
                        lns[sb_] = stage_stats(zs[sb_], rs.pop(sb_))
                    if pt_b is not None and not MASK_EARLY:
                        ptcs[b] = stage_mask(pt_b, ins[b][2])

            if repeat == 1:
                body()
            else:
                with tc.For_i(0, repeat, 1):
                    body()

    nc.finalize()
    return nc


_NC_CACHE = {}


def _get_nc():
    if "nc" not in _NC_CACHE:
        _NC_CACHE["nc"] = build_nc()
    return _NC_CACHE["nc"]


def prep_in_maps(inputs, mask_array, dw1, dw2, dw3, db1, db2, db3):
    X = np.ascontiguousarray(
        np.asarray(inputs, dtype=np.float32).reshape(
            BATCH * BLOCK_NUM, BLOCK_LEN, DIM))
    m = np.asarray(mask_array, dtype=np.float32).reshape(
        BATCH * BLOCK_NUM, BLOCK_LEN, DIM)
    nb = BATCH * BLOCK_NUM

    # X^T variants --------------------------------------------------------
    xt = X.transpose(0, 2, 1)                      # [b, d, t]
    xt8 = (xt * np.float32(1.0 / C_X)).astype(NP_F8)
    # xt8int[b, p, i, tw, c, j] = xt8[b, 128*(2i+j)+p, tw*128 + (127-c)]
    tmp = xt8.reshape(nb, 2, 2, NC_P, NCH, NC_P)   # [b, i, j, p, tw, c]
    x8i = np.ascontiguousarray(
        tmp[:, :, :, :, :, ::-1].transpose(0, 3, 1, 4, 5, 2))
    # x8p[b, p, dc, t]: plain X^T/2 for the contiguous A-moving read
    x8p = np.ascontiguousarray(
        xt8.reshape(nb, NCH, NC_P, BLOCK_LEN).transpose(0, 2, 1, 3))
    # xtb[b, p, dc, k] = bf16 X[b, k, dc*128+p]
    xtb = np.ascontiguousarray(
        xt.reshape(nb, NCH, NC_P, BLOCK_LEN).transpose(0, 2, 1, 3)
    ).astype(NP_BF16)
    xn_nat = X.reshape(nb, NCH, NC_P, DIM)
    mT = m.transpose(0, 2, 1)                      # [b, k, q]
    if USE_X8P:
        # at free axis natural -> pt q natural -> O partitions reversed:
        # reverse xn rows per window, un-reverse out rows on the host
        xn = np.ascontiguousarray(
            xn_nat[:, :, ::-1, :].transpose(0, 2, 1, 3)).astype(NP_BF16)
        m8s = mT
    else:
        # interleaved A-read reverses at's free axis; reversals cancel at O
        xn = np.ascontiguousarray(
            xn_nat.transpose(0, 2, 1, 3)).astype(NP_BF16)
        m8s = mT.reshape(nb, DIM, NCH, NC_P)[:, :, :, ::-1].reshape(
            nb, DIM, BLOCK_LEN)
    m8 = np.ascontiguousarray(
        m8s.reshape(nb, NCH, NC_P, BLOCK_LEN).transpose(0, 2, 1, 3)
    ).astype(NP_F8)

    # weights -------------------------------------------------------------
    scale = np.float32(C_W12 / math.sqrt(DIM))
    w12 = ((np.asarray(dw1, np.float32) @ np.asarray(dw2, np.float32).T)
           * scale).astype(NP_F8)
    # w12int[p, i, d2w, c, j] = w12[128*(2i+j)+p, d2w*128 + (127-c)]
    t2 = np.asarray(w12).reshape(2, 2, NC_P, NCH, NC_P)  # [i, j, p, d2w, c]
    w12i = np.ascontiguousarray(
        t2[:, :, :, :, ::-1].transpose(2, 0, 3, 4, 1))
    w3 = np.ascontiguousarray(
        (np.asarray(dw3, np.float32) * np.float32(C_X))
        .reshape(NCH, NC_P, DIM).transpose(1, 0, 2)).astype(NP_F8)

    in_maps = []
    for c in range(N_CORES):
        s = slice(c * NBLK, (c + 1) * NBLK)
        im = {"x8i": x8i[s], "xtb": xtb[s],
              "xn": xn[s], "m8": m8[s], "w12i": w12i, "w3": w3}
        if USE_X8P:
            im["x8p"] = x8p[s]
        in_maps.append(im)
    return in_maps


def kernel(inputs, mask_array, dw1, dw2, dw3, db1, db2, db3):
    nc = _get_nc()
    in_maps = prep_in_maps(inputs, mask_array, dw1, dw2, dw3, db1, db2, db3)
    res = run_bass_kernel_spmd(nc, in_maps, list(range(N_CORES)))
    out = np.concatenate(
        [np.asarray(res.results[c]["out"]) for c in range(N_CORES)], axis=0)
    out = out.astype(np.float32)
    if USE_X8P:
        out = out[:, ::-1, :, :]
    out = out.transpose(0, 2, 1, 3).reshape(
        BATCH, BLOCK_NUM, BLOCK_LEN, DIM)
    return np.ascontiguousarray(out)
